# revision 9
# baseline (speedup 1.0000x reference)
"""Trainium2 Bass kernel for nn_MCILayer (Mamba-style MCI layer), v2.

Data-parallel over batch (8 elements -> 8 cores). Per core: the 4096-step
concat sequence as 2 chunks (x, xi) x 4 folds of 512 steps packed on the
partition axis for all pointwise stages. Restructured phase C: B/C scan
inputs are produced by direct matmuls from h (no compact->expand chain),
alpha/u_s prep for a whole chunk lands in chunk-wide [128,2048] bf16
fields, the selective scan runs as ONE tensor_tensor_scan per (chunk,
d-half), and the y reduction + output projection happen in a post pass.
GpSimd carries PSUM drains / products / half the residual adds. All
mamba-path intermediates are bf16 (the residual path stays exact fp32).

Self-contained: hardcodes shapes from the problem spec.
"""
import os

os.environ.setdefault("NEURON_RT_LOG_LEVEL", "WARNING")

import numpy as np

DIM, Bz, L = 768, 8, 2048
DR, DI, DS, K = 8, 16, 16, 4
T = 2 * L                  # concat length per batch element = 4096
NCH = 2                    # sequence chunks (x-half, xi-half)
TC = T // NCH              # 2048 timesteps per chunk
F = 4                      # folds per chunk
TF = TC // F               # 512 timesteps per fold
NG = 4                     # dma groups per chunk (1 per fold)


def _consts_from_weights(W):
    """Host-side packing of weights into kernel tile layouts."""
    f32 = np.float32
    W_in = W["W_in"].astype(f32)                     # [8, 32]
    conv_w = W["conv_w"].reshape(DI, K).astype(f32)  # [16, 4]
    conv_b = W["conv_b"].astype(f32)
    W_xp = W["W_xp"].astype(f32)                     # [16, 33]
    W_dt = W["W_dt"].astype(f32)                     # [1, 16]
    b_dt = W["b_dt"].astype(f32)
    A = -np.exp(W["A_log"].astype(np.float64)).astype(f32)   # [16, 16]
    Dp = W["Dp"].astype(f32)
    W_out = W["W_out"].astype(f32)                   # [16, 8]
    W_ix = W["W_ix"].astype(f32)
    W_ixi = W["W_ixi"].astype(f32)
    b_in = W["b_in"].astype(f32)                     # [32]

    for nm in ("b_dx", "b_dxi", "b_out", "b_ix", "b_ixi"):
        assert np.abs(W[nm]).max() == 0.0, f"{nm} must be zero"
    assert np.abs(b_in[:DI]).max() == 0.0, "b_in h-part must be zero"

    c = {}
    # inproj weights: per (ch, ct, f): [128, 32] with W col k at col f*8+k
    wdsf = np.zeros((128, 2 * 6 * F * 32), f32)
    for ch, Wd in enumerate((W["W_dx"].astype(f32), W["W_dxi"].astype(f32))):
        for ct in range(6):
            for f in range(F):
                off = ((ch * 6 + ct) * F + f) * 32
                wdsf[:, off + f*8: off + f*8 + 8] = Wd[ct*128:(ct+1)*128, :]
    c["wdsf"] = wdsf

    # uP [32,512] -> hz [128,512]: rows f*32+i (h, i<16), f*32+16+i (z)
    w4hz = np.zeros((32, 128), f32)
    # uP -> z on rows f*32..f*32+16 (aligned with y rows for gating)
    wz2 = np.zeros((32, 128), f32)
    for f in range(F):
        w4hz[f*8:(f+1)*8, f*32:(f+1)*32] = W_in
        wz2[f*8:(f+1)*8, f*32:f*32+DI] = W_in[:, DI:]
    c["w4hz"], c["wz2"] = w4hz, wz2

    # h -> dt rows (f, 0..16)
    W_hdt = W_xp[:, 0:1] @ W_dt                      # [16, 16]
    wdt2 = np.zeros((128, 128), f32)
    for f in range(F):
        wdt2[f*32:f*32+DI, f*32:f*32+DI] = W_hdt
    c["wdt2"] = wdt2

    # h -> expanded B/C fields: out row p gets B(s(p)) / C(s(p)), per fold
    for f in range(F):
        wbx = np.zeros((128, 128), f32)
        wcx = np.zeros((128, 128), f32)
        for p in range(128):
            s = p % 16
            for i in range(DI):
                wbx[f*32 + i, p] = W_xp[i, 1 + s]
                wcx[f*32 + i, p] = W_xp[i, 1 + DS + s]
        c[f"wbx{f}"] = wbx
        c[f"wcx{f}"] = wcx

    # dt/dth expansion and y reduction, per (f, hh): d(p) = hh*8 + p//16
    for f in range(F):
        for hh in range(2):
            ed = np.zeros((128, 128), f32)
            ry = np.zeros((128, 128), f32)
            for p in range(128):
                d = hh * 8 + p // 16
                ed[f*32 + d, p] = 1.0
                ry[p, f*32 + d] = 1.0
            c[f"edf{f}{hh}"] = ed
            c[f"ryf{f}{hh}"] = ry

    # out proj: gated rows (f, i) -> oS rows f*8+k
    wout4 = np.zeros((128, 32), f32)
    for f in range(F):
        wout4[f*32:f*32+DI, f*8:f*8+DR] = W_out
    c["wout4"] = wout4

    # final: per-fold [32, 2*DIM], scaled x32 for fp8; x | xi halves
    for f in range(F):
        wf = np.zeros((32, 2*DIM), f32)
        wf[f*8:f*8+DR, 0:DIM] = W_ix * 32.0
        wf[f*8:f*8+DR, DIM:2*DIM] = W_ixi * 32.0
        c[f"wfin{f}"] = wf

    c["ident"] = np.eye(128, dtype=f32)
    c["ident8k"] = np.eye(128, dtype=f32) * 8192.0

    # per-partition scalars: conv taps 0-3, conv_b, b_dt, Dp
    ppc = np.zeros((128, 8), f32)
    for f in range(F):
        r0 = f * 32
        ppc[r0:r0+DI, 0:4] = conv_w
        ppc[r0:r0+DI, 4] = conv_b
        ppc[r0:r0+DI, 5] = b_dt
        ppc[r0+DI:r0+32, 5] = b_dt
        ppc[r0:r0+DI, 7] = Dp
    # A scale per (d,s) row for each hh
    for hh in range(2):
        for p in range(128):
            ppc[p, 6] += 0.0   # keep col 6 free/zero
    apc = np.zeros((128, 2), f32)
    for hh in range(2):
        for p in range(128):
            apc[p, hh] = A[hh*8 + p // 16, p % 16]
    c["ppc"] = ppc
    c["apc"] = apc
    return c


CONST_SHAPES = {"wdsf": (128, 2*6*F*32), "w4hz": (32, 128), "wz2": (32, 128),
                "wdt2": (128, 128), "wout4": (128, 32),
                "ident": (128, 128), "ident8k": (128, 128),
                "ppc": (128, 8), "apc": (128, 2)}
CONST8_SHAPES = {}
for _f in range(F):
    CONST8_SHAPES[f"wfin{_f}"] = (32, 2*DIM)
CONST8_OFF = {}
_o8 = 0
for _n in CONST8_SHAPES:
    CONST8_OFF[_n] = _o8
    _o8 += CONST8_SHAPES[_n][1]
CST8_W = _o8


def pack_cstack8(c):
    import ml_dtypes
    out = np.zeros((128, CST8_W), ml_dtypes.float8_e4m3fn)
    for n in CONST8_SHAPES:
        rows, cols = CONST8_SHAPES[n]
        out[:rows, CONST8_OFF[n]:CONST8_OFF[n]+cols] = \
            c[n].astype(ml_dtypes.float8_e4m3fn)
    return out
for _f in range(F):
    CONST_SHAPES[f"wbx{_f}"] = (128, 128)
    CONST_SHAPES[f"wcx{_f}"] = (128, 128)
    for _hh in range(2):
        CONST_SHAPES[f"edf{_f}{_hh}"] = (128, 128)
        CONST_SHAPES[f"ryf{_f}{_hh}"] = (128, 128)

# f32 consts (ident f32r; ppc/apc f32); everything else bf16
CONST_F32 = {"ident", "ident8k", "ppc", "apc"}
CONST_ORDER = [n for n in CONST_SHAPES if n in CONST_F32]
CONST16_ORDER = [n for n in CONST_SHAPES if n not in CONST_F32]
CONST_OFF = {}
_off = 0
for _n in CONST_ORDER:
    CONST_OFF[_n] = _off
    _off += CONST_SHAPES[_n][1]
CST_W = _off
_off = 0
for _n in CONST16_ORDER:
    CONST_OFF[_n] = _off
    _off += CONST_SHAPES[_n][1]
CST16_W = _off


def pack_cstack(c):
    out = np.zeros((128, CST_W), np.float32)
    for n in CONST_ORDER:
        rows, cols = CONST_SHAPES[n]
        out[:rows, CONST_OFF[n]:CONST_OFF[n]+cols] = c[n]
    return out


def pack_cstack16(c):
    import ml_dtypes
    out = np.zeros((128, CST16_W), ml_dtypes.bfloat16)
    for n in CONST16_ORDER:
        rows, cols = CONST_SHAPES[n]
        out[:rows, CONST_OFF[n]:CONST_OFF[n]+cols] = c[n].astype(ml_dtypes.bfloat16)
    return out


def build_bass():
    import concourse.bacc as bacc
    import concourse.tile as tile
    from concourse import mybir

    f32 = mybir.dt.float32
    f32r = mybir.dt.float32r
    bf16 = mybir.dt.bfloat16
    AF = mybir.ActivationFunctionType
    OP = mybir.AluOpType

    nc = bacc.Bacc()
    xcat = nc.dram_tensor("xcat", [T, DIM], f32r, kind="ExternalInput")
    out_d = nc.dram_tensor("out", [T, DIM], f32, kind="ExternalOutput")
    cstack_d = nc.dram_tensor("cstack", [128, CST_W], f32r, kind="ExternalInput")
    cstack16_d = nc.dram_tensor("cstack16", [128, CST16_W], bf16,
                                kind="ExternalInput")
    fp8 = mybir.dt.float8e4
    cstack8_d = nc.dram_tensor("cstack8", [128, CST8_W], fp8,
                               kind="ExternalInput")

    with tile.TileContext(nc) as tc:
        with (
            tc.tile_pool(name="consts", bufs=1) as cp,
            tc.tile_pool(name="xnat", bufs=8) as xp,
            tc.tile_pool(name="xt", bufs=3) as xtp,
            tc.tile_pool(name="small", bufs=2) as smp,
            tc.tile_pool(name="fields", bufs=2) as fp,
            tc.tile_pool(name="bespool", bufs=3) as besp,
            tc.tile_pool(name="prodp", bufs=3) as prp,
            tc.tile_pool(name="ogp", bufs=4) as ogp,
            tc.tile_pool(name="persist", bufs=1) as pp,
            tc.tile_pool(name="rot", bufs=3, space="PSUM") as ps,
            tc.tile_pool(name="acc", bufs=1, space="PSUM") as accp,
        ):
            cstack = cp.tile([128, CST_W], f32r, tag="cstack")
            nc.sync.dma_start(cstack[:], cstack_d[:])
            cstack16 = cp.tile([128, CST16_W], bf16, tag="cstack16")
            nc.sync.dma_start(cstack16[:], cstack16_d[:])
            cstack8 = cp.tile([128, CST8_W], fp8, tag="cstack8")
            nc.sync.dma_start(cstack8[:], cstack8_d[:])

            def C8(n):
                rows, cols = CONST8_SHAPES[n]
                return cstack8[0:rows, CONST8_OFF[n]:CONST8_OFF[n]+cols]

            def CW(n):
                rows, cols = CONST_SHAPES[n]
                return cstack16[0:rows, CONST_OFF[n]:CONST_OFF[n]+cols]

            def CV(n, rows=None):
                r, cols = CONST_SHAPES[n]
                if rows is not None:
                    r = rows
                ap = cstack[0:r, CONST_OFF[n]:CONST_OFF[n]+cols]
                if n not in ("ident", "ident8k"):
                    ap = ap.bitcast(f32)
                return ap

            ppc = CV("ppc")
            apc = CV("apc")
            identr = CV("ident")

            halo = pp.tile([32, 4], f32, tag="halo")
            nc.vector.memset(halo[:], 0.0)

            # pre-sync: each engine observes the const DMAs before real work
            scr_ps = ps.tile([128, 128], f32r, tag="rot")
            nc.tensor.transpose(scr_ps[:], identr, identr)
            scr_sb = cp.tile([128, 4], f32, tag="scr")
            nc.scalar.copy(scr_sb[:, 0:1], ppc[:, 0:1])
            nc.vector.tensor_copy(scr_sb[:, 1:2], ppc[:, 0:1])
            scr16 = ps.tile([128, 128], f32, tag="rot")
            nc.tensor.matmul(scr16[0:32, 0:128], CW("wout4")[:, 0:32],
                             CW("wdt2")[:, 0:128], start=True, stop=True)

            # ---- all input DMAs up front (queues stream in issue order)
            xnat = []
            for g in range(2 * NG):
                xg = xp.tile([128, 4 * DIM], f32r, tag="xnat")
                src = xcat[g*512:(g+1)*512, :]
                src = src.rearrange("(jj p) c -> p jj c", p=128)
                nc.sync.dma_start(
                    xg[:].rearrange("p (jj c) -> p jj c", jj=4), src)
                xnat.append(xg)

            # rotate PSUM->SBUF drains: scalar x2 : vector x1 (gpsimd
            # cannot access PSUM)
            drain_rr = [0]

            def drain(dst, src):
                e = drain_rr[0] % 3
                drain_rr[0] += 1
                if e < 2:
                    nc.scalar.copy(dst, src)
                else:
                    nc.vector.tensor_copy(dst, src)

            # per-chunk state shared between phases
            St = {}      # (ch, hh) -> chunk-wide scan output tile
            hT = {}      # ch -> h tile
            zsiT = {}    # ch -> silu(z)
            ddtT = {}    # ch -> [dt | dth]
            AlT = {}     # (ch, hh) -> alpha field
            UsT = {}     # (ch, hh) -> scan input field
            oST = {}     # ch -> output-proj compact
            ypT = {}     # ch -> y accumulation psum

            def phase_ABprep(ch):
                # -------- phase A: transpose + inproj -> uP
                upp = accp.tile([32, TF], f32, tag="upp")
                first = True
                for f in range(F):
                    for ct in range(6):
                        tp = ps.tile([128, 512], f32r, tag="rot")
                        for jj in range(4):
                            nc.tensor.transpose(
                                tp[:, jj*128:(jj+1)*128],
                                xnat[ch*NG + f][:, jj*DIM + ct*128:
                                                jj*DIM + (ct+1)*128],
                                identr)
                        xT = xtp.tile([128, 512], bf16, tag="xt")
                        drain(xT[:], tp[:])
                        woff = ((ch * 6 + ct) * F + f) * 32
                        nc.tensor.matmul(
                            upp[:], CW("wdsf")[:, woff:woff+32], xT[:],
                            start=first, stop=(f == F-1 and ct == 5))
                        first = False
                uP = smp.tile([32, TF], bf16, tag="uP")
                nc.scalar.copy(uP[:], upp[:])

                # -------- phase B
                hzp = ps.tile([128, TF], f32, tag="rot")
                nc.tensor.matmul(hzp[:], CW("w4hz"), uP[:], start=True,
                                 stop=True)
                ext = smp.tile([128, TF + 3], bf16, tag="ext")
                nc.scalar.copy(ext[:, 3:3+TF], hzp[:])
                for f in range(1, F):
                    nc.scalar.copy(ext[f*32:(f+1)*32, 0:3],
                                   hzp[(f-1)*32:f*32, TF-3:TF])
                nc.scalar.copy(ext[0:32, 0:3], halo[:, 0:3])
                nc.scalar.copy(halo[:, 0:3], hzp[96:128, TF-3:TF])

                cacc = smp.tile([128, TF], bf16, tag="cacc")
                nc.vector.tensor_scalar_mul(cacc[:], ext[:, 0:TF], ppc[:, 0:1])
                for k in range(1, K):
                    cacc2 = smp.tile([128, TF], bf16, tag="cacc")
                    nc.vector.scalar_tensor_tensor(
                        cacc2[:], ext[:, k:k+TF], ppc[:, k:k+1], cacc[:],
                        op0=OP.mult, op1=OP.add)
                    cacc = cacc2
                h = smp.tile([128, TF], bf16, tag="h")
                nc.scalar.activation(h[:], cacc[:], AF.Silu, bias=ppc[:, 4:5])
                hT[ch] = h

                z2p = ps.tile([128, TF], f32, tag="rot")
                nc.tensor.matmul(z2p[:], CW("wz2"), uP[:],
                                 start=True, stop=True)
                zsi = smp.tile([128, TF], bf16, tag="zsi")
                nc.scalar.activation(zsi[:], z2p[:], AF.Silu)
                zsiT[ch] = zsi

                dtp = ps.tile([128, TF], f32, tag="rot")
                nc.tensor.matmul(dtp[:], CW("wdt2"), h[:], start=True,
                                 stop=True)
                dte_ = smp.tile([128, TF], bf16, tag="dte_")
                nc.scalar.activation(dte_[:], dtp[:], AF.Exp, bias=ppc[:, 5:6])
                ddt = smp.tile([128, 2 * TF], bf16, tag="ddt")
                nc.scalar.activation(ddt[:, 0:TF], dte_[:], AF.Ln, bias=1.0)
                nc.vector.tensor_mul(ddt[:, TF:2*TF], ddt[:, 0:TF], h[:])
                ddtT[ch] = ddt

                # -------- phase C prep: expansions + alpha + u_s
                Al = [fp.tile([128, TC], bf16, tag=f"Al{hh}", name=f"Al{hh}")
                      for hh in range(2)]
                Us = [fp.tile([128, TC], bf16, tag=f"Us{hh}", name=f"Us{hh}")
                      for hh in range(2)]
                for f in range(F):
                    bep = ps.tile([128, TF], f32, tag="rot")
                    nc.tensor.matmul(bep[:], CW(f"wbx{f}"), h[:], start=True,
                                     stop=True)
                    bes = besp.tile([128, TF], bf16, tag="bes")
                    nc.scalar.copy(bes[:], bep[:])
                    for hh in range(2):
                        dte = ps.tile([128, TF], f32, tag="rot")
                        nc.tensor.matmul(dte[:], CW(f"edf{f}{hh}"),
                                         ddt[:, 0:TF], start=True, stop=True)
                        nc.scalar.activation(Al[hh][:, f*TF:(f+1)*TF], dte[:],
                                             AF.Exp, scale=apc[:, hh:hh+1])
                        dthe = ps.tile([128, TF], f32, tag="rot")
                        nc.tensor.matmul(dthe[:], CW(f"edf{f}{hh}"),
                                         ddt[:, TF:2*TF], start=True, stop=True)
                        nc.vector.tensor_mul(Us[hh][:, f*TF:(f+1)*TF],
                                             dthe[:], bes[:])
                AlT[(ch, 0)], AlT[(ch, 1)] = Al
                UsT[(ch, 0)], UsT[(ch, 1)] = Us

                # -------- scans: one per (chunk, hh), chained chunk to chunk
                for hh in range(2):
                    S = fp.tile([128, TC], bf16, tag=f"S{hh}")
                    init = 0.0 if ch == 0 else St[(ch - 1, hh)][:, TC-1:TC]
                    nc.vector.tensor_tensor_scan(
                        S[:], Al[hh][:], Us[hh][:], init,
                        op0=OP.mult, op1=OP.add)
                    St[(ch, hh)] = S

            def phase_postD(ch):
                h, zsi = hT[ch], zsiT[ch]
                # -------- phase C post: prod + y reduction
                yp = accp.tile([128, TF], f32, tag="yp")
                for f in range(F):
                    cep = ps.tile([128, TF], f32, tag="rot")
                    nc.tensor.matmul(cep[:], CW(f"wcx{f}"), h[:], start=True,
                                     stop=True)
                    for hh in range(2):
                        prod = prp.tile([128, TF], bf16, tag="prod")
                        nc.vector.tensor_mul(
                            prod[:], St[(ch, hh)][:, f*TF:(f+1)*TF], cep[:])
                        nc.tensor.matmul(yp[:], CW(f"ryf{f}{hh}"), prod[:],
                                         start=(f == 0 and hh == 0),
                                         stop=(f == F-1 and hh == 1))

                # -------- phase D: gate + out-proj + residual + store
                tmp = smp.tile([128, TF], bf16, tag="tmp")
                nc.vector.scalar_tensor_tensor(
                    tmp[:], h[:], ppc[:, 7:8], yp[:], op0=OP.mult, op1=OP.add)
                gated = smp.tile([128, TF], bf16, tag="gated")
                nc.vector.tensor_mul(gated[:], tmp[:], zsi[:])

                opp = ps.tile([32, TF], f32, tag="rot")
                nc.tensor.matmul(opp[:], CW("wout4"), gated[:],
                                 start=True, stop=True)
                oS = smp.tile([32, TF], fp8, tag="oS")
                nc.scalar.activation(oS[:], opp[:], AF.Copy, scale=256.0)

                for j in range(16):
                    f, cq = j // 4, j % 4
                    g, jj = j // 4, j % 4
                    og = ogp.tile([128, DIM], f32, tag="og")
                    xop = ps.tile([128, DIM], f32, tag="rot")
                    xsl = xnat[ch*NG + g][:, jj*DIM:(jj+1)*DIM]
                    on_scalar = (j % 4 != 3)
                    for cs, cw in ((0, 512), (512, 256)):
                        nc.tensor.matmul(
                            xop[:, cs:cs+cw], oS[:, cq*128:(cq+1)*128],
                            C8(f"wfin{f}")[:, ch*DIM + cs: ch*DIM + cs + cw],
                            start=True, stop=not on_scalar)
                        if on_scalar:
                            nc.tensor.matmul(xop[:, cs:cs+cw], CV("ident8k"),
                                             xsl[:, cs:cs+cw],
                                             start=False, stop=True)
                    if on_scalar:
                        nc.scalar.activation(og[:], xop[:], AF.Copy,
                                             scale=1.0 / 8192.0)
                    else:
                        nc.vector.scalar_tensor_tensor(
                            og[:], xop[:], 1.0 / 8192.0, xsl.bitcast(f32),
                            op0=OP.mult, op1=OP.add)
                    dd = out_d[ch*TC + j*128: ch*TC + (j+1)*128, :]
                    nc.sync.dma_start(dd, og[:])

            phase_ABprep(0)
            phase_ABprep(1)
            phase_postD(0)
            phase_postD(1)

    nc.compile()
    return nc


_CACHE = {}


def kernel(**inputs):
    inputs = {k: np.ascontiguousarray(np.asarray(v, dtype=np.float32))
              if np.asarray(v).dtype != np.int32 else np.asarray(v)
              for k, v in inputs.items()}
    x, xi = inputs["x"], inputs["xi"]
    W = {k: v for k, v in inputs.items() if k not in ("x", "xi")}
    consts = _consts_from_weights(W)

    if "nc" not in _CACHE:
        _CACHE["nc"] = build_bass()
    nc = _CACHE["nc"]

    from concourse.bass_utils import run_bass_kernel_spmd
    cstack = pack_cstack(consts)
    cstack16 = pack_cstack16(consts)
    cstack8 = pack_cstack8(consts)
    in_maps = []
    for b in range(Bz):
        m = {"cstack": cstack, "cstack16": cstack16, "cstack8": cstack8,
             "xcat": np.ascontiguousarray(np.concatenate([x[b], xi[b]], axis=0))}
        in_maps.append(m)
    res = run_bass_kernel_spmd(nc, in_maps, core_ids=list(range(Bz)),
                               **_CACHE.get("run_kwargs", {}))
    _CACHE["last_res"] = res
    x_out = np.stack([res.results[b]["out"][:L] for b in range(Bz)])
    xi_out = np.stack([res.results[b]["out"][L:] for b in range(Bz)])
    return (x_out, xi_out)


# revision 14
# speedup vs baseline: 1.2618x; 1.2618x over previous
"""Trainium2 Bass kernel for nn_MCILayer (Mamba-style MCI layer), v3.

Data-parallel over batch (8 elements -> 8 cores). Per core: the 4096-step
concat sequence as 2 chunks (x, xi) x 4 folds of 512 steps packed on the
partition axis for all pointwise stages.

v3: the input projection consumes an fp8 copy of x delivered PRE-TRANSPOSED
by the DMA transpose XBAR (2-byte path on uint16 views), so the PE never
transposes and no PSUM->SBUF xT drains exist; the projection runs as fp8
DoubleRow matmuls (K=256 per instruction). Phase C produces scan inputs
with direct matmuls from h into chunk-wide [128,2048] bf16 fields, one
tensor_tensor_scan per (chunk, d-half), y reduction + output projection in
a post pass. The residual add rides the PE (scaled-identity matmul into
the projection PSUM) for scalar-drained outputs. All mamba-path
intermediates are bf16/fp8 (the residual path stays exact fp32).

Self-contained: hardcodes shapes from the problem spec.
"""
import os

os.environ.setdefault("NEURON_RT_LOG_LEVEL", "WARNING")

import numpy as np

DIM, Bz, L = 768, 8, 2048
DR, DI, DS, K = 8, 16, 16, 4
T = 2 * L                  # concat length per batch element = 4096
NCH = 2                    # sequence chunks (x-half, xi-half)
TC = T // NCH              # 2048 timesteps per chunk
F = 4                      # folds per chunk
TF = TC // F               # 512 timesteps per fold
NG = 4                     # f32 dma groups per chunk (1 per fold)
NCB = 3                    # c-pair blocks (768 channels = 3 x (128 pairs))
WSC = 32.0                 # fp8 weight scale (inproj + final)
OSC = 256.0                # fp8 oS scale


def _consts_from_weights(W):
    """Host-side packing of weights into kernel tile layouts."""
    f32 = np.float32
    W_in = W["W_in"].astype(f32)                     # [8, 32]
    conv_w = W["conv_w"].reshape(DI, K).astype(f32)  # [16, 4]
    conv_b = W["conv_b"].astype(f32)
    W_xp = W["W_xp"].astype(f32)                     # [16, 33]
    W_dt = W["W_dt"].astype(f32)                     # [1, 16]
    b_dt = W["b_dt"].astype(f32)
    A = -np.exp(W["A_log"].astype(np.float64)).astype(f32)   # [16, 16]
    Dp = W["Dp"].astype(f32)
    W_out = W["W_out"].astype(f32)                   # [16, 8]
    W_ix = W["W_ix"].astype(f32)
    W_ixi = W["W_ixi"].astype(f32)
    b_in = W["b_in"].astype(f32)                     # [32]

    for nm in ("b_dx", "b_dxi", "b_out", "b_ix", "b_ixi"):
        assert np.abs(W[nm]).max() == 0.0, f"{nm} must be zero"
    assert np.abs(b_in[:DI]).max() == 0.0, "b_in h-part must be zero"

    c = {}
    # inproj weights (fp8 DoubleRow): per (ch, cb, f) block [128, 2, 32]:
    # w8[q, lo, f*8+k] = Wd[2*(cb*128+q)+lo, k] * WSC
    wdsf8 = np.zeros((128, 2 * NCB * F, 2, 32), f32)
    for ch, Wd in enumerate((W["W_dx"].astype(f32), W["W_dxi"].astype(f32))):
        for cb in range(NCB):
            for f in range(F):
                blk = (ch * NCB + cb) * F + f
                for lo in range(2):
                    crows = Wd[2*cb*128 + lo: 2*(cb+1)*128: 2, :]  # c = 2q+lo
                    wdsf8[:, blk, lo, f*8:f*8+8] = crows * WSC
    c["wdsf8"] = wdsf8.reshape(128, 2 * NCB * F * 64)

    # uP [32,512] -> hz [128,512]: rows f*32+i (h, i<16), f*32+16+i (z)
    w4hz = np.zeros((32, 128), f32)
    wz2 = np.zeros((32, 128), f32)
    for f in range(F):
        w4hz[f*8:(f+1)*8, f*32:(f+1)*32] = W_in
        wz2[f*8:(f+1)*8, f*32:f*32+DI] = W_in[:, DI:]
    c["w4hz"], c["wz2"] = w4hz, wz2

    # h -> dt rows (f, 0..16)
    W_hdt = W_xp[:, 0:1] @ W_dt                      # [16, 16]
    wdt2 = np.zeros((128, 128), f32)
    for f in range(F):
        wdt2[f*32:f*32+DI, f*32:f*32+DI] = W_hdt
    c["wdt2"] = wdt2

    # h -> expanded B/C fields: out row p gets B(s(p)) / C(s(p)), per fold
    for f in range(F):
        wbx = np.zeros((128, 128), f32)
        wcx = np.zeros((128, 128), f32)
        for p in range(128):
            s = p % 16
            for i in range(DI):
                wbx[f*32 + i, p] = W_xp[i, 1 + s]
                wcx[f*32 + i, p] = W_xp[i, 1 + DS + s]
        c[f"wbx{f}"] = wbx
        c[f"wcx{f}"] = wcx

    # dt/dth expansion and y reduction, per (f, hh): d(p) = hh*8 + p//16
    for f in range(F):
        for hh in range(2):
            ed = np.zeros((128, 128), f32)
            ry = np.zeros((128, 128), f32)
            for p in range(128):
                d = hh * 8 + p // 16
                ed[f*32 + d, p] = 1.0
                ry[p, f*32 + d] = 1.0
            c[f"edf{f}{hh}"] = ed
            c[f"ryf{f}{hh}"] = ry

    # out proj: gated rows (f, i) -> oS rows f*8+k
    wout4 = np.zeros((128, 32), f32)
    for f in range(F):
        wout4[f*32:f*32+DI, f*8:f*8+DR] = W_out
    c["wout4"] = wout4

    # final: per-fold [32, 2*DIM] fp8, scaled x32; x | xi halves
    for f in range(F):
        wf = np.zeros((32, 2*DIM), f32)
        wf[f*8:f*8+DR, 0:DIM] = W_ix * WSC
        wf[f*8:f*8+DR, DIM:2*DIM] = W_ixi * WSC
        c[f"wfin{f}"] = wf

    c["ident8k"] = np.eye(128, dtype=f32) * (WSC * OSC)

    # per-partition scalars: conv taps 0-3, conv_b, b_dt, (free), Dp
    ppc = np.zeros((128, 8), f32)
    for f in range(F):
        r0 = f * 32
        ppc[r0:r0+DI, 0:4] = conv_w
        ppc[r0:r0+DI, 4] = conv_b
        ppc[r0:r0+DI, 5] = b_dt
        ppc[r0+DI:r0+32, 5] = b_dt
        ppc[r0:r0+DI, 7] = Dp
    apc = np.zeros((128, 2), f32)
    for hh in range(2):
        for p in range(128):
            apc[p, hh] = A[hh*8 + p // 16, p % 16]
    c["ppc"] = ppc
    c["apc"] = apc
    return c


CONST_SHAPES = {"w4hz": (32, 128), "wz2": (32, 128),
                "wdt2": (128, 128), "wout4": (128, 32),
                "ident8k": (128, 128), "ppc": (128, 8), "apc": (128, 2)}
for _f in range(F):
    CONST_SHAPES[f"wbx{_f}"] = (128, 128)
    CONST_SHAPES[f"wcx{_f}"] = (128, 128)
    for _hh in range(2):
        CONST_SHAPES[f"edf{_f}{_hh}"] = (128, 128)
        CONST_SHAPES[f"ryf{_f}{_hh}"] = (128, 128)

CONST_F32 = {"ident8k", "ppc", "apc"}
CONST_ORDER = [n for n in CONST_SHAPES if n in CONST_F32]
CONST16_ORDER = [n for n in CONST_SHAPES if n not in CONST_F32]
CONST_OFF = {}
_off = 0
for _n in CONST_ORDER:
    CONST_OFF[_n] = _off
    _off += CONST_SHAPES[_n][1]
CST_W = _off
_off = 0
for _n in CONST16_ORDER:
    CONST_OFF[_n] = _off
    _off += CONST_SHAPES[_n][1]
CST16_W = _off

CONST8_SHAPES = {"wdsf8": (128, 2 * NCB * F * 64)}
for _f in range(F):
    CONST8_SHAPES[f"wfin{_f}"] = (32, 2*DIM)
CONST8_OFF = {}
_o8 = 0
for _n in CONST8_SHAPES:
    CONST8_OFF[_n] = _o8
    _o8 += CONST8_SHAPES[_n][1]
CST8_W = _o8


def pack_cstack(c):
    out = np.zeros((128, CST_W), np.float32)
    for n in CONST_ORDER:
        rows, cols = CONST_SHAPES[n]
        out[:rows, CONST_OFF[n]:CONST_OFF[n]+cols] = c[n]
    return out


def pack_cstack16(c):
    import ml_dtypes
    out = np.zeros((128, CST16_W), ml_dtypes.bfloat16)
    for n in CONST16_ORDER:
        rows, cols = CONST_SHAPES[n]
        out[:rows, CONST_OFF[n]:CONST_OFF[n]+cols] = c[n].astype(ml_dtypes.bfloat16)
    return out


def pack_cstack8(c):
    import ml_dtypes
    out = np.zeros((128, CST8_W), ml_dtypes.float8_e4m3fn)
    for n in CONST8_SHAPES:
        rows, cols = CONST8_SHAPES[n]
        out[:rows, CONST8_OFF[n]:CONST8_OFF[n]+cols] = \
            c[n].astype(ml_dtypes.float8_e4m3fn)
    return out


def build_bass():
    import concourse.bacc as bacc
    import concourse.tile as tile
    from concourse import mybir

    f32 = mybir.dt.float32
    f32r = mybir.dt.float32r
    bf16 = mybir.dt.bfloat16
    fp8 = mybir.dt.float8e4
    u16 = mybir.dt.uint16
    AF = mybir.ActivationFunctionType
    OP = mybir.AluOpType
    PM = mybir.MatmulPerfMode

    nc = bacc.Bacc()
    xcat = nc.dram_tensor("xcat", [T, DIM], f32r, kind="ExternalInput")
    xcat8 = nc.dram_tensor("xcat8", [T, DIM // 2], u16, kind="ExternalInput")
    out_d = nc.dram_tensor("out", [T, DIM], f32, kind="ExternalOutput")
    cstack_d = nc.dram_tensor("cstack", [128, CST_W], f32r, kind="ExternalInput")
    cstack16_d = nc.dram_tensor("cstack16", [128, CST16_W], bf16,
                                kind="ExternalInput")
    cstack8_d = nc.dram_tensor("cstack8", [128, CST8_W], fp8,
                               kind="ExternalInput")

    with tile.TileContext(nc) as tc:
        with (
            tc.tile_pool(name="consts", bufs=1) as cp,
            tc.tile_pool(name="xnat", bufs=8) as xp,
            tc.tile_pool(name="xt8", bufs=6) as xtp,
            tc.tile_pool(name="small", bufs=2) as smp,
            tc.tile_pool(name="fields", bufs=2) as fp,
            tc.tile_pool(name="bespool", bufs=2) as besp,
            tc.tile_pool(name="prodp", bufs=2) as prp,
            tc.tile_pool(name="ogp", bufs=3) as ogp,
            tc.tile_pool(name="persist", bufs=1) as pp,
            tc.tile_pool(name="rot", bufs=6, space="PSUM") as ps,
            tc.tile_pool(name="acc", bufs=1, space="PSUM") as accp,
        ):
            cstack = cp.tile([128, CST_W], f32r, tag="cstack")
            nc.sync.dma_start(cstack[:], cstack_d[:])
            cstack16 = cp.tile([128, CST16_W], bf16, tag="cstack16")
            nc.sync.dma_start(cstack16[:], cstack16_d[:])
            cstack8 = cp.tile([128, CST8_W], fp8, tag="cstack8")
            nc.sync.dma_start(cstack8[:], cstack8_d[:])

            def CW(n):
                rows, cols = CONST_SHAPES[n]
                return cstack16[0:rows, CONST_OFF[n]:CONST_OFF[n]+cols]

            def CV(n):
                rows, cols = CONST_SHAPES[n]
                ap = cstack[0:rows, CONST_OFF[n]:CONST_OFF[n]+cols]
                if n != "ident8k":
                    ap = ap.bitcast(f32)
                return ap

            def C8(n):
                rows, cols = CONST8_SHAPES[n]
                return cstack8[0:rows, CONST8_OFF[n]:CONST8_OFF[n]+cols]

            ppc = CV("ppc")
            apc = CV("apc")

            halo = pp.tile([32, 4], f32, tag="halo")
            nc.vector.memset(halo[:], 0.0)

            # pre-sync: each engine observes the const DMAs before real work
            scr_ps = ps.tile([128, 128], f32, tag="rot")
            nc.tensor.matmul(scr_ps[0:32, 0:128], CW("wout4")[:, 0:32],
                             CW("wdt2")[:, 0:128], start=True, stop=True)
            scr_sb = cp.tile([128, 4], f32, tag="scr")
            nc.scalar.copy(scr_sb[:, 0:1], ppc[:, 0:1])
            nc.vector.tensor_copy(scr_sb[:, 1:2], ppc[:, 0:1])

            # ---- input DMAs: per chunk, fp8-transposed blocks first, then
            # the f32 residual groups (queues stream in issue order)
            xT8 = {}
            xnat = []
            for ch in range(NCH):
                for cb in range(NCB):
                    xt = xtp.tile([128, TC], u16, tag="xt8",
                                  name=f"xt8_{ch}{cb}")
                    nc.sync.dma_start(
                        xt[:], xcat8[ch*TC:(ch+1)*TC, cb*128:(cb+1)*128],
                        transpose=True)
                    xT8[(ch, cb)] = xt
                for g in range(NG):
                    xg = xp.tile([128, 4 * DIM], f32r, tag="xnat",
                                 name=f"xnat_{ch}{g}")
                    src = xcat[(ch*NG + g)*512:(ch*NG + g + 1)*512, :]
                    src = src.rearrange("(jj p) c -> p jj c", p=128)
                    nc.sync.dma_start(
                        xg[:].rearrange("p (jj c) -> p jj c", jj=4), src)
                    xnat.append(xg)

            St = {}
            hT = {}
            zsiT = {}

            def phase_ABprep(ch):
                # -------- phase A: fp8 DoubleRow inproj from transposed x
                upp = accp.tile([32, TF], f32, tag="upp")
                first = True
                for f in range(F):
                    for cb in range(NCB):
                        blk = (ch * NCB + cb) * F + f
                        w8 = C8("wdsf8")[:, blk*64:(blk+1)*64]
                        w8 = w8.rearrange("p (two m) -> p two m", two=2)
                        rhs = xT8[(ch, cb)][:].bitcast(fp8)
                        rhs = rhs.rearrange("p (t two) -> p two t", two=2)
                        rhs = rhs[:, :, f*TF:(f+1)*TF]
                        nc.tensor.matmul(
                            upp[:], w8, rhs, perf_mode=PM.DoubleRow,
                            start=first, stop=(f == F-1 and cb == NCB-1))
                        first = False
                uP = smp.tile([32, TF], bf16, tag="uP")
                nc.scalar.activation(uP[:], upp[:], AF.Copy, scale=1.0 / WSC)

                # -------- phase B
                hzp = ps.tile([128, TF], f32, tag="rot")
                nc.tensor.matmul(hzp[:], CW("w4hz"), uP[:], start=True,
                                 stop=True)
                ext = smp.tile([128, TF + 3], bf16, tag="ext")
                nc.scalar.copy(ext[:, 3:3+TF], hzp[:])
                for f in range(1, F):
                    nc.scalar.copy(ext[f*32:(f+1)*32, 0:3],
                                   hzp[(f-1)*32:f*32, TF-3:TF])
                nc.scalar.copy(ext[0:32, 0:3], halo[:, 0:3])
                nc.scalar.copy(halo[:, 0:3], hzp[96:128, TF-3:TF])

                cacc = smp.tile([128, TF], bf16, tag="cacc")
                nc.vector.tensor_scalar_mul(cacc[:], ext[:, 0:TF], ppc[:, 0:1])
                for k in range(1, K):
                    cacc2 = smp.tile([128, TF], bf16, tag="cacc")
                    nc.vector.scalar_tensor_tensor(
                        cacc2[:], ext[:, k:k+TF], ppc[:, k:k+1], cacc[:],
                        op0=OP.mult, op1=OP.add)
                    cacc = cacc2
                h = smp.tile([128, TF], bf16, tag="h")
                nc.scalar.activation(h[:], cacc[:], AF.Silu, bias=ppc[:, 4:5])
                hT[ch] = h

                z2p = ps.tile([128, TF], f32, tag="rot")
                nc.tensor.matmul(z2p[:], CW("wz2"), uP[:], start=True,
                                 stop=True)
                zsi = smp.tile([128, TF], bf16, tag="zsi")
                nc.scalar.activation(zsi[:], z2p[:], AF.Silu)
                zsiT[ch] = zsi

                dtp = ps.tile([128, TF], f32, tag="rot")
                nc.tensor.matmul(dtp[:], CW("wdt2"), h[:], start=True,
                                 stop=True)
                ddt = smp.tile([128, 2 * TF], bf16, tag="ddt")
                nc.scalar.activation(ddt[:, TF:2*TF], dtp[:], AF.Exp,
                                     bias=ppc[:, 5:6])
                nc.scalar.activation(ddt[:, 0:TF], ddt[:, TF:2*TF], AF.Ln,
                                     bias=1.0)
                nc.vector.tensor_mul(ddt[:, TF:2*TF], ddt[:, 0:TF], h[:])

                # -------- phase C prep: expansions + alpha + u_s
                Al = [fp.tile([128, TC], bf16, tag=f"Al{hh}", name=f"Al{hh}")
                      for hh in range(2)]
                Us = [fp.tile([128, TC], bf16, tag=f"Us{hh}", name=f"Us{hh}",
                              bufs=1)
                      for hh in range(2)]
                for f in range(F):
                    bep = ps.tile([128, TF], f32, tag="rot")
                    nc.tensor.matmul(bep[:], CW(f"wbx{f}"), h[:], start=True,
                                     stop=True)
                    bes = besp.tile([128, TF], bf16, tag="bes")
                    nc.scalar.copy(bes[:], bep[:])
                    for hh in range(2):
                        dte = ps.tile([128, TF], f32, tag="rot")
                        nc.tensor.matmul(dte[:], CW(f"edf{f}{hh}"),
                                         ddt[:, 0:TF], start=True, stop=True)
                        nc.scalar.activation(Al[hh][:, f*TF:(f+1)*TF], dte[:],
                                             AF.Exp, scale=apc[:, hh:hh+1])
                        dthe = ps.tile([128, TF], f32, tag="rot")
                        nc.tensor.matmul(dthe[:], CW(f"edf{f}{hh}"),
                                         ddt[:, TF:2*TF], start=True, stop=True)
                        nc.vector.tensor_mul(Us[hh][:, f*TF:(f+1)*TF],
                                             dthe[:], bes[:])

                # -------- scans: one per (chunk, hh), chained chunk to chunk
                for hh in range(2):
                    S = fp.tile([128, TC], bf16, tag=f"S{hh}", name=f"S{hh}")
                    init = 0.0 if ch == 0 else St[(ch - 1, hh)][:, TC-1:TC]
                    nc.vector.tensor_tensor_scan(
                        S[:], Al[hh][:], Us[hh][:], init,
                        op0=OP.mult, op1=OP.add)
                    St[(ch, hh)] = S

            def phase_postD(ch):
                h, zsi = hT[ch], zsiT[ch]
                # -------- phase C post: prod + y reduction
                yp = accp.tile([128, TF], f32, tag="yp")
                for f in range(F):
                    cep = ps.tile([128, TF], f32, tag="rot")
                    nc.tensor.matmul(cep[:], CW(f"wcx{f}"), h[:], start=True,
                                     stop=True)
                    for hh in range(2):
                        prod = prp.tile([128, TF], bf16, tag="prod")
                        nc.vector.tensor_mul(
                            prod[:], St[(ch, hh)][:, f*TF:(f+1)*TF], cep[:])
                        nc.tensor.matmul(yp[:], CW(f"ryf{f}{hh}"), prod[:],
                                         start=(f == 0 and hh == 0),
                                         stop=(f == F-1 and hh == 1))

                # -------- phase D: gate + out-proj + residual + store
                tmp = smp.tile([128, TF], bf16, tag="tmp")
                nc.vector.scalar_tensor_tensor(
                    tmp[:], h[:], ppc[:, 7:8], yp[:], op0=OP.mult, op1=OP.add)
                gated = smp.tile([128, TF], bf16, tag="gated")
                nc.vector.tensor_mul(gated[:], tmp[:], zsi[:])

                opp = ps.tile([32, TF], f32, tag="rot")
                nc.tensor.matmul(opp[:], CW("wout4"), gated[:],
                                 start=True, stop=True)
                oS = smp.tile([32, TF], fp8, tag="oS")
                nc.scalar.activation(oS[:], opp[:], AF.Copy, scale=OSC)

                for j in range(16):
                    f, cq = j // 4, j % 4
                    g, jj = j // 4, j % 4
                    og = ogp.tile([128, DIM], f32, tag="og")
                    xsl = xnat[ch*NG + g][:, jj*DIM:(jj+1)*DIM]
                    on_scalar = (j % 4 != 3)
                    for cs, cw in ((0, 512), (512, 256)):
                        xop = ps.tile([128, 512], f32, tag="rot")
                        nc.tensor.matmul(
                            xop[:, 0:cw], oS[:, cq*128:(cq+1)*128],
                            C8(f"wfin{f}")[:, ch*DIM + cs: ch*DIM + cs + cw],
                            start=True, stop=not on_scalar)
                        if on_scalar:
                            nc.tensor.matmul(xop[:, 0:cw], CV("ident8k"),
                                             xsl[:, cs:cs+cw],
                                             start=False, stop=True)
                            nc.scalar.activation(og[:, cs:cs+cw], xop[:, 0:cw],
                                                 AF.Copy,
                                                 scale=1.0 / (WSC * OSC))
                        else:
                            nc.vector.scalar_tensor_tensor(
                                og[:, cs:cs+cw], xop[:, 0:cw],
                                1.0 / (WSC * OSC),
                                xsl[:, cs:cs+cw].bitcast(f32),
                                op0=OP.mult, op1=OP.add)
                    dd = out_d[ch*TC + j*128: ch*TC + (j+1)*128, :]
                    nc.sync.dma_start(dd, og[:])

            phase_ABprep(0)
            phase_ABprep(1)
            phase_postD(0)
            phase_postD(1)

    nc.compile()
    return nc


_CACHE = {}


def kernel(**inputs):
    import ml_dtypes
    inputs = {k: np.ascontiguousarray(np.asarray(v, dtype=np.float32))
              if np.asarray(v).dtype != np.int32 else np.asarray(v)
              for k, v in inputs.items()}
    x, xi = inputs["x"], inputs["xi"]
    W = {k: v for k, v in inputs.items() if k not in ("x", "xi")}
    consts = _consts_from_weights(W)

    if "nc" not in _CACHE:
        _CACHE["nc"] = build_bass()
    nc = _CACHE["nc"]

    from concourse.bass_utils import run_bass_kernel_spmd
    cstack = pack_cstack(consts)
    cstack16 = pack_cstack16(consts)
    cstack8 = pack_cstack8(consts)
    in_maps = []
    for b in range(Bz):
        xc = np.ascontiguousarray(np.concatenate([x[b], xi[b]], axis=0))
        xc8 = xc.astype(ml_dtypes.float8_e4m3fn).view(np.uint16)
        m = {"cstack": cstack, "cstack16": cstack16, "cstack8": cstack8,
             "xcat": xc, "xcat8": xc8}
        in_maps.append(m)
    res = run_bass_kernel_spmd(nc, in_maps, core_ids=list(range(Bz)),
                               **_CACHE.get("run_kwargs", {}))
    _CACHE["last_res"] = res
    x_out = np.stack([res.results[b]["out"][:L] for b in range(Bz)])
    xi_out = np.stack([res.results[b]["out"][L:] for b in range(Bz)])
    return (x_out, xi_out)
